# revision 46
# baseline (speedup 1.0000x reference)
"""Self-contained TRN2 kernel for the bidirectional attention correction.

kernel(hl, hr) -> (mu_lr, mu_rl), matching:
    hl_n = rownorm(hl); hr_n = rownorm(hr)
    a = hl_n @ hr_n.T
    mu_lr = hr_n - softmax(a, 1).T @ hl_n
    mu_rl = hl_n - softmax(a, 0) @ hr_n

Runs SPMD on 8 NeuronCores: core c owns rows [c*1024,(c+1)*1024) of hl and
hr. The builder below constructs one Bass/Tile graph shared by all cores.
"""

import sys

for _p in ("/opt/trn_rl_repo",):
    if _p not in sys.path:
        sys.path.insert(0, _p)

from contextlib import ExitStack

import numpy as np

import concourse.bass as bass
import concourse.tile as tile
from concourse import bacc, mybir
from concourse.masks import make_identity
from concourse.tile import add_dep_helper

F32 = mybir.dt.float32
BF16 = mybir.dt.bfloat16
FP8 = mybir.dt.float8e4

ADD = mybir.AluOpType.add
SUB = mybir.AluOpType.subtract
MULT = mybir.AluOpType.mult
BYPASS = mybir.AluOpType.bypass
EXP = mybir.ActivationFunctionType.Exp
COPY = mybir.ActivationFunctionType.Copy
AXL_X = mybir.AxisListType.X
DROW = mybir.MatmulPerfMode.DoubleRow


def build(C=8, NL=1024, M=8192, D=1024, stop_after="full"):
    """Build + compile the SPMD Bass graph."""
    PB = NL // 128          # local row blocks (i)
    DK = D // 128           # contraction chunks over D
    JB = M // 128           # j 128-blocks
    BLK = M // C            # j-cols per gather block (== NL here)
    NLH = NL // 2           # j-cols per gather half
    W1 = min(512, NLH)      # P1 j-chunk width
    NQ = NLH // W1          # chunks per (half, block) piece
    JC = M // W1            # P1 j-chunks
    DW = min(512, D)        # d-chunk width for P2 outputs
    DH = D // DW            # d-halves
    G = 2                   # P2b i-blocks per sweep
    NG = (PB + G - 1) // G
    S1 = float(8 * M)       # hl' fp8 scale
    S2 = float(M // 2)      # p0T fp8 scale
    SV = float(2 * 8 * M)   # vlr fp8 scale (P2a partials for the RS)
    SL = 16.0               # hl_n.T fp8 scale (P1 lhsT)
    SR = 16.0               # hr_n.T fp8 scale (P1 rhs)
    groups = [list(range(C))]
    LVL = {"prep": 0, "p1": 1, "p2a": 2, "p2b": 3, "full": 4}[stop_after]
    assert PB % 2 == 0

    nc = bacc.Bacc("TRN2", target_bir_lowering=False, debug=False, num_devices=C)

    hl_in = nc.dram_tensor("hl", [NL, D], F32, kind="ExternalInput").ap()
    hr_in = nc.dram_tensor("hr", [NL, D], F32, kind="ExternalInput").ap()
    mu_lr_o = nc.dram_tensor("mu_lr", [NL, D], F32, kind="ExternalOutput").ap()
    mu_rl_o = nc.dram_tensor("mu_rl", [NL, D], F32, kind="ExternalOutput").ap()

    with tile.TileContext(nc) as tc, ExitStack() as ctx:
        dram = ctx.enter_context(tc.tile_pool(name="dram", bufs=1, space="DRAM"))
        sb = ctx.enter_context(tc.tile_pool(name="sb", bufs=1))
        ps = ctx.enter_context(tc.tile_pool(name="ps", bufs=1, space="PSUM"))

        # ---- internal DRAM ----
        hrnT_loc = [dram.tile([D, NLH], FP8, name=f"hrnT_loc{h}")
                    for h in range(2)]
        hrn8_loc = dram.tile([NL, D], FP8)
        hrnT_all = [dram.tile([C, D, NLH], FP8, name=f"hrnT_all{h}",
                              addr_space="Shared") for h in range(2)]
        hrn8_all = dram.tile([C, BLK, D], FP8, addr_space="Shared")
        s_loc = dram.tile([M], F32)
        s_glob = dram.tile([M], F32, addr_space="Shared")
        vlr = dram.tile([M, D], FP8)
        vred = dram.tile([NL, D], FP8)

        # ---- SBUF resident ----
        exp_a = sb.tile([128, PB, M], FP8, name="exp_a")       # exp(a) rows
        hl_nb = sb.tile([128, PB, D], BF16, name="hl_nb")      # hl_n
        hl_nT = sb.tile([128, DK, NL], FP8, name="hl_nT")      # hl_n.T*SL
        hrn_b = sb.tile([128, PB, D], BF16, name="hrn_b")      # hr_n local
        hlp8 = sb.tile([128, PB, D], FP8, name="hlp8")         # hl'*S1 fp8
        hrn8c_st = sb.tile([128, 1, D], FP8, name="hrn8c_st")  # fp8 cast stage
        # streaming / staging (manual rotation via slot dims)
        rhsT = sb.tile([128, 4, DK, W1], FP8, name="rhsT")     # P1 rhs stream
        p0T_c = sb.tile([128, JB, G, 128], FP8, name="p0T_c")  # P2b lhsT cache
        vlr_st = sb.tile([128, 5, D], FP8, name="vlr_st")
        out_st = sb.tile([128, 3, DW], F32, name="out_st")
        vred_st = sb.tile([128, 2, DW], F32, name="vred_st")
        fin_st = sb.tile([128, 2, DW], F32, name="fin_st")
        trT_st = sb.tile([128, 2, DK, 128], FP8, name="trT_st")
        ld_st = sb.tile([128, 2, D], F32, name="ld_st")
        s_row = sb.tile([1, 1, W1], F32, name="s_row")
        # consts / stats
        ident_b = sb.tile([128, 128], BF16, name="ident_b")
        ident_e = sb.tile([128, 128], FP8, name="ident_e")
        ones_e = sb.tile([128, 2, 16], FP8, name="ones_e")
        stats = sb.tile([128, 352], F32, name="stats")
        r_parts = stats[:, 0:PB * JC].rearrange("p (a b) -> p a b", a=PB)
        r_red = stats[:, 128:128 + PB]
        r_red3 = stats[:, 128:128 + PB].rearrange("p (a b) -> p a b", b=1)
        rinv = stats[:, 136:136 + PB]
        s_sb = stats[:, 144:208][:, :JB]
        srec = stats[:, 208:272][:, :JB]
        sinv = stats[:, 272:336][:, :JB]
        nrm = stats[:, 336:352].rearrange("p (a b) -> p a b", a=8)  # [128,8,2]

        make_identity(nc, ident_b)
        nc.vector.tensor_copy(out=ident_e, in_=ident_b)
        nc.vector.memset(ones_e, 1.0)

        # ======= prep: hr first (gathers launch early), then hl ============
        SQUARE = mybir.ActivationFunctionType.Square

        def norm_chunk(src, dstb, pb, it):
            ld = ld_st[:, it % 2, :]
            nc.sync.dma_start(out=ld, in_=src[pb * 128:(pb + 1) * 128, :])
            nm = nrm[:, it % 8, :]
            for dd in range(DH):
                sq = ps.tile([128, DW], F32, tag="rot", bufs=3,
                             name=f"sq{it}_{dd}")
                nc.scalar.activation(out=sq, in_=ld[:, dd * DW:(dd + 1) * DW],
                                     func=SQUARE, accum_out=nm[:, dd:dd + 1])
            if DH > 1:
                nc.vector.tensor_add(out=nm[:, 0:1], in0=nm[:, 0:1],
                                     in1=nm[:, 1:2])
            nc.scalar.sqrt(out=nm[:, 1:2], in_=nm[:, 0:1])
            nc.vector.reciprocal(out=nm[:, 0:1], in_=nm[:, 1:2])
            nc.vector.tensor_scalar_mul(
                out=dstb[:, pb, :], in0=ld, scalar1=nm[:, 0:1])

        # interleaved prep: hr chunk (feeds gathers) then hl chunk — hl's
        # PE transposes fill the hr norm-chain latency gaps
        def hr_chunk(pb):
            h, pq = divmod(pb, PB // 2)
            norm_chunk(hr_in, hrn_b, pb, 2 * pb)
            tstage = trT_st[:, pb % 2, :, :]
            for dk in range(DK):
                pst = ps.tile([128, 128], BF16, tag="rot", bufs=3,
                              name=f"ptB{pb}_{dk}")
                nc.tensor.transpose(
                    pst, hrn_b[:, pb, dk * 128:(dk + 1) * 128], ident_b)
                nc.scalar.mul(out=tstage[:, dk, :], in_=pst, mul=SR)
            nc.sync.dma_start(
                out=hrnT_loc[h].rearrange("(dk p) j -> p dk j", p=128)
                [:, :, pq * 128:(pq + 1) * 128],
                in_=tstage)

        def hl_chunk(pb):
            norm_chunk(hl_in, hl_nb, pb, 2 * pb + 1)
            for dk in range(DK):
                pst = ps.tile([128, 128], BF16, tag="rot", bufs=3,
                              name=f"ptA{pb}_{dk}")
                nc.tensor.transpose(
                    pst, hl_nb[:, pb, dk * 128:(dk + 1) * 128], ident_b)
                nc.vector.tensor_scalar_mul(
                    out=hl_nT[:, dk, pb * 128:(pb + 1) * 128], in0=pst,
                    scalar1=SL)

        for pb in range(PB // 2):
            hr_chunk(pb)
        nc.gpsimd.collective_compute(
            "AllGather", BYPASS, replica_groups=groups,
            ins=[hrnT_loc[0].opt()], outs=[hrnT_all[0].opt()])
        for pb in range(PB // 2):
            hr_chunk(PB // 2 + pb)
            hl_chunk(2 * pb)
            hl_chunk(2 * pb + 1)
        nc.gpsimd.collective_compute(
            "AllGather", BYPASS, replica_groups=groups,
            ins=[hrnT_loc[1].opt()], outs=[hrnT_all[1].opt()])
        # hr_n fp8 -> DRAM + gather
        hrn8_rows = hrn8_loc.rearrange("(pb p) d -> p pb d", p=128)
        for pb in range(PB):
            t8 = hrn8c_st[:, 0, :]
            nc.gpsimd.tensor_copy(out=t8, in_=hrn_b[:, pb, :])
            nc.sync.dma_start(out=hrn8_rows[:, pb, :], in_=t8)
        nc.gpsimd.collective_compute(
            "AllGather", BYPASS, replica_groups=groups,
            ins=[hrn8_loc.opt()], outs=[hrn8_all.opt()])

        # ================= P1: a-matmul + exp + row/col sums ================
        # j-chunk order follows gather halves: (h, b, q)
        chunks = [(h, b, q) for h in range(2) for b in range(C)
                  for q in range(NQ)]
        if LVL >= 1:
            deferred = []

            def flush_deferred():
                while deferred:
                    deferred.pop(0)()

            psum_s = [None, None]

            for ci, (h, b, q) in enumerate(chunks):
                j0 = b * BLK + h * NLH + q * W1   # global j of this chunk
                rt = rhsT[:, ci % 4, :, :]
                nc.sync.dma_start(
                    out=rt,
                    in_=hrnT_all[h][b].rearrange("(dk p) j -> p dk j", p=128)
                    [:, :, q * W1:(q + 1) * W1])
                for ib in range(PB):
                    t_ = ci * PB + ib
                    pa = ps.tile([128, W1], F32,
                                 tag=("rot" if t_ % 7 < 3 else "acc"),
                                 bufs=(3 if t_ % 7 < 3 else 4),
                                 name=f"pa{ci}_{ib}")
                    for dkp in range(DK // 2):
                        nc.tensor.matmul(
                            pa,
                            lhsT=hl_nT[:, 2 * dkp:2 * dkp + 2,
                                       ib * 128:(ib + 1) * 128],
                            rhs=rt[:, 2 * dkp:2 * dkp + 2, :],
                            start=(dkp == 0), stop=(dkp == DK // 2 - 1),
                            perf_mode=DROW)
                    et = exp_a[:, ib, j0:j0 + W1]
                    nc.scalar.activation(
                        out=et, in_=pa, func=EXP, scale=1.0 / (SL * SR),
                        accum_out=r_parts[:, ib, ci:ci + 1])
                    if ib % 2 == 0:
                        flush_deferred()
                        continue

                    def ones_mm(ci=ci, ib=ib, j0=j0):
                        ibp = ib // 2
                        if ibp == 0:
                            psum_s[0] = ps.tile([1, W1], F32, tag="colsum",
                                                bufs=1, name=f"pscs{ci}")
                        # DoubleRow column-sum over an i-pair
                        nc.tensor.matmul(
                            psum_s[0], lhsT=ones_e[:, :, 0:1],
                            rhs=exp_a[:, ib - 1:ib + 1, j0:j0 + W1],
                            start=(ibp == 0), stop=(ibp == PB // 2 - 1),
                            perf_mode=DROW)
                        if ib == PB - 1:
                            sr = s_row[:, 0, :]
                            nc.vector.tensor_copy(out=sr, in_=psum_s[0])
                            nc.sync.dma_start(
                                out=s_loc[j0:j0 + W1].rearrange(
                                    "(a b) -> a b", a=1),
                                in_=sr)

                    flush_deferred()
                    deferred.append(ones_mm)
            flush_deferred()

            # r -> S1/r ; hl' fp8
            nc.vector.tensor_reduce(out=r_red3, in_=r_parts, op=ADD, axis=AXL_X)
            nc.vector.reciprocal(out=rinv, in_=r_red)
            for ib in range(PB):
                nc.vector.tensor_scalar(
                    out=hlp8[:, ib, :], in0=hl_nb[:, ib, :],
                    scalar1=rinv[:, ib:ib + 1], scalar2=S1, op0=MULT, op1=MULT)

            # col sums: AllReduce + readback in [p, jb] layout
            nc.gpsimd.collective_compute(
                "AllReduce", ADD, replica_groups=groups,
                ins=[s_loc.opt()], outs=[s_glob.opt()])
            nc.sync.dma_start(
                out=s_sb, in_=s_glob.rearrange("(b p) -> p b", p=128))
            nc.vector.reciprocal(out=srec, in_=s_sb)
            nc.vector.tensor_scalar_mul(out=sinv, in0=srec, scalar1=S2)

        # ================= P2a: v_lr = exp_a.T @ hl' -> ReduceScatter =======
        # P2b rhs kept resident per d-half; shares the rhsT slot (dead
        # after P1). Loaded once per half: 8 MB total instead of 32 MB
        # re-streamed, so P2b needs no DMA while the ReduceScatters run.
        rhs_res = sb.tile([128, JB, DW], FP8, name="rhs_res", tag="rhsT")

        def load_rhs_half(dh, engine=None):
            eng = engine or nc.gpsimd
            for b in range(C):
                ld_i = eng.dma_start(
                    out=rhs_res[:, b * (BLK // 128):(b + 1) * (BLK // 128), :],
                    in_=hrn8_all[b].rearrange("(jb p) d -> p jb d", p=128)
                    [:, :, dh * DW:(dh + 1) * DW])
                last_reload[0] = ld_i

        def transpose_grp(jb, g):
            ibs = list(range(g * G, min((g + 1) * G, PB)))
            for xi, ib in enumerate(ibs):
                # fp8 transpose: output element step must be 2
                pst = ps.tile([128, 128, 2], FP8, tag="rot", bufs=3,
                              name=f"ptb{g}_{jb}_{xi}")
                nc.tensor.transpose(
                    pst[:, :, 0],
                    exp_a[:, ib, jb * 128:(jb + 1) * 128], ident_e)
                if (jb * G + xi) % 3 == 2:
                    nc.scalar.activation(
                        out=p0T_c[:, jb, xi, :], in_=pst[:, :, 0],
                        func=COPY, scale=sinv[:, jb:jb + 1])
                else:
                    nc.vector.tensor_scalar_mul(
                        out=p0T_c[:, jb, xi, :], in0=pst[:, :, 0],
                        scalar1=sinv[:, jb:jb + 1])

        last_add = [None]
        last_add_dh = {}
        last_reload = [None]
        if LVL >= 2:
            if LVL >= 3:
                load_rhs_half(0)  # runs behind P2a's compute
            for jb in range(JB):
                st = vlr_st[:, jb % 5, :]
                pls = [ps.tile([128, DW], F32,
                               tag=("rot" if dh == 0 else "acc"),
                               bufs=(3 if dh == 0 else 4),
                               name=f"pl{jb}_{dh}") for dh in range(DH)]
                for icp in range(PB // 2):
                    for dh in range(DH):
                        # DoubleRow: contraction over an i-pair (K=256);
                        # dh-inner so consecutive matmuls share the lhsT load
                        nc.tensor.matmul(
                            pls[dh],
                            lhsT=exp_a[:, 2 * icp:2 * icp + 2,
                                       jb * 128:(jb + 1) * 128],
                            rhs=hlp8[:, 2 * icp:2 * icp + 2,
                                     dh * DW:(dh + 1) * DW],
                            start=(icp == 0), stop=(icp == PB // 2 - 1),
                            perf_mode=DROW)
                for dh in range(DH):
                    nc.scalar.activation(out=st[:, dh * DW:(dh + 1) * DW],
                                         in_=pls[dh], func=COPY,
                                         scale=SV / S1)
                nc.scalar.dma_start(
                    out=vlr[jb * 128:(jb + 1) * 128, :], in_=st)
            # single ReduceScatter runs on the CC engine while P2b
            # (which needs no DMA) computes
            nc.gpsimd.collective_compute(
                "ReduceScatter", ADD, replica_groups=groups,
                ins=[vlr.opt()], outs=[vred.opt()])

        # ====== P2b: mu_rl = hl_n - (p0T.T @ hr_n)/S2  (sweeps dh x g) ======
        if LVL >= 3:
            for dh in range(DH):
                if dh > 0:
                    # HW lanes: per-b-block deps let dh1 sweeps start as
                    # soon as the first block lands
                    load_rhs_half(dh, engine=nc.sync)
                for g in range(NG):
                    ibs = list(range(g * G, min((g + 1) * G, PB)))
                    accs = [ps.tile([128, DW], F32, tag="acc", bufs=4,
                                    name=f"acc{g}_{dh}_{xi}")
                            for xi in range(len(ibs))]
                    if dh == 0:
                        for jb in range(4):
                            transpose_grp(jb, g)
                    for jbp in range(JB // 2):
                        if dh == 0:
                            for jb in (2 * jbp + 4, 2 * jbp + 5):
                                if jb < JB:
                                    transpose_grp(jb, g)
                        for xi in range(len(ibs)):
                            # DoubleRow: contraction over a jb-pair (K=256)
                            nc.tensor.matmul(
                                accs[xi],
                                lhsT=p0T_c[:, 2 * jbp:2 * jbp + 2, xi, :],
                                rhs=rhs_res[:, 2 * jbp:2 * jbp + 2, :],
                                start=(jbp == 0), stop=(jbp == JB // 2 - 1),
                                perf_mode=DROW)
                    for xi, ib in enumerate(ibs):
                        st = out_st[:, ((dh * NG + g) * G + xi) % 3, :DW]
                        nc.scalar.activation(
                            out=st, in_=accs[xi], func=COPY, scale=-1.0 / S2)
                        last_add[0] = nc.vector.tensor_add(
                            out=st, in0=st,
                            in1=hl_nb[:, ib, dh * DW:(dh + 1) * DW])
                        last_add_dh[dh] = last_add[0]
                        nc.gpsimd.dma_start(
                            out=mu_rl_o[ib * 128:(ib + 1) * 128,
                                        dh * DW:(dh + 1) * DW], in_=st)

        # ================= final: mu_lr = hr_n - vred (on gpsimd) ===========
        if LVL >= 4:
            for dh in range(DH):
                for pb in range(PB):
                    vs = vred_st[:, pb % 2, :]
                    rd_i = nc.gpsimd.dma_start(
                        out=vs,
                        in_=vred[pb * 128:(pb + 1) * 128,
                                 dh * DW:(dh + 1) * DW])
                    if last_reload[0] is not None:
                        add_dep_helper(rd_i.ins, last_reload[0].ins,
                                       sync=False, reason="after reload")
                    st = fin_st[:, pb % 2, :]
                    nc.gpsimd.tensor_scalar_mul(
                        out=st, in0=vs, scalar1=-1.0 / SV)
                    nc.gpsimd.tensor_add(
                        out=st, in0=st,
                        in1=hrn_b[:, pb, dh * DW:(dh + 1) * DW])
                    nc.sync.dma_start(
                        out=mu_lr_o[pb * 128:(pb + 1) * 128,
                                    dh * DW:(dh + 1) * DW], in_=st)

        # dummy writes for any output a stopped-early build didn't produce
        if LVL < 4:
            for pb in range(PB):
                for dd in range(DH):
                    st = out_st[:, pb % 3, :]
                    nc.vector.tensor_copy(
                        out=st, in_=hrn_b[:, pb, dd * DW:(dd + 1) * DW])
                    nc.sync.dma_start(
                        out=mu_lr_o[pb * 128:(pb + 1) * 128,
                                    dd * DW:(dd + 1) * DW], in_=st)
        if LVL < 3:
            for pb in range(PB):
                for dh in range(DH):
                    st = fin_st[:, pb % 2, :]
                    nc.vector.tensor_copy(
                        out=st, in_=hl_nb[:, pb, dh * DW:(dh + 1) * DW])
                    nc.sync.dma_start(
                        out=mu_rl_o[pb * 128:(pb + 1) * 128,
                                    dh * DW:(dh + 1) * DW], in_=st)

    nc.compile()
    return nc


_NC_CACHE = {}


def _get_nc():
    if "nc" not in _NC_CACHE:
        _NC_CACHE["nc"] = build(C=8, NL=1024, M=8192, D=1024)
    return _NC_CACHE["nc"]


def kernel(hl, hr):
    """Full inputs in, full outputs out; distributes across 8 cores."""
    from concourse.bass_utils import run_bass_kernel_spmd

    C, NL = 8, 1024
    hl = np.ascontiguousarray(np.asarray(hl, dtype=np.float32))
    hr = np.ascontiguousarray(np.asarray(hr, dtype=np.float32))
    nc = _get_nc()
    in_maps = [
        {"hl": np.ascontiguousarray(hl[c * NL:(c + 1) * NL]),
         "hr": np.ascontiguousarray(hr[c * NL:(c + 1) * NL])}
        for c in range(C)
    ]
    res = run_bass_kernel_spmd(nc, in_maps, list(range(C)))
    mu_lr = np.concatenate([res.results[c]["mu_lr"] for c in range(C)])
    mu_rl = np.concatenate([res.results[c]["mu_rl"] for c in range(C)])
    return mu_lr, mu_rl



# revision 47
# speedup vs baseline: 1.1366x; 1.1366x over previous
"""Self-contained TRN2 kernel for the bidirectional attention correction.

kernel(hl, hr) -> (mu_lr, mu_rl), matching:
    hl_n = rownorm(hl); hr_n = rownorm(hr)
    a = hl_n @ hr_n.T
    mu_lr = hr_n - softmax(a, 1).T @ hl_n
    mu_rl = hl_n - softmax(a, 0) @ hr_n

Runs SPMD on 8 NeuronCores: core c owns rows [c*1024,(c+1)*1024) of hl and
hr. The builder below constructs one Bass/Tile graph shared by all cores.
"""

import sys

for _p in ("/opt/trn_rl_repo",):
    if _p not in sys.path:
        sys.path.insert(0, _p)

from contextlib import ExitStack

import numpy as np

import concourse.bass as bass
import concourse.tile as tile
from concourse import bacc, mybir
from concourse.masks import make_identity
from concourse.tile import add_dep_helper

F32 = mybir.dt.float32
BF16 = mybir.dt.bfloat16
FP8 = mybir.dt.float8e4

ADD = mybir.AluOpType.add
SUB = mybir.AluOpType.subtract
MULT = mybir.AluOpType.mult
BYPASS = mybir.AluOpType.bypass
EXP = mybir.ActivationFunctionType.Exp
COPY = mybir.ActivationFunctionType.Copy
AXL_X = mybir.AxisListType.X
DROW = mybir.MatmulPerfMode.DoubleRow


def build(C=8, NL=1024, M=8192, D=1024, stop_after="full"):
    """Build + compile the SPMD Bass graph."""
    PB = NL // 128          # local row blocks (i)
    DK = D // 128           # contraction chunks over D
    JB = M // 128           # j 128-blocks
    BLK = M // C            # j-cols per gather block (== NL here)
    NLH = NL // 2           # j-cols per gather half
    W1 = min(512, NLH)      # P1 j-chunk width
    NQ = NLH // W1          # chunks per (half, block) piece
    JC = M // W1            # P1 j-chunks
    DW = min(512, D)        # d-chunk width for P2 outputs
    DH = D // DW            # d-halves
    G = 2                   # P2b i-blocks per sweep
    NG = (PB + G - 1) // G
    S1 = float(8 * M)       # hl' fp8 scale
    S2 = float(M // 2)      # p0T fp8 scale
    SV = float(2 * 8 * M)   # vlr fp8 scale (P2a partials for the RS)
    SL = 16.0               # hl_n.T fp8 scale (P1 lhsT)
    SR = 16.0               # hr_n.T fp8 scale (P1 rhs)
    groups = [list(range(C))]
    LVL = {"prep": 0, "p1": 1, "p2a": 2, "p2b": 3, "full": 4}[stop_after]
    assert PB % 2 == 0

    nc = bacc.Bacc("TRN2", target_bir_lowering=False, debug=False, num_devices=C)

    hl_in = nc.dram_tensor("hl", [NL, D], F32, kind="ExternalInput").ap()
    hr_in = nc.dram_tensor("hr", [NL, D], F32, kind="ExternalInput").ap()
    mu_lr_o = nc.dram_tensor("mu_lr", [NL, D], F32, kind="ExternalOutput").ap()
    mu_rl_o = nc.dram_tensor("mu_rl", [NL, D], F32, kind="ExternalOutput").ap()

    with tile.TileContext(nc) as tc, ExitStack() as ctx:
        dram = ctx.enter_context(tc.tile_pool(name="dram", bufs=1, space="DRAM"))
        sb = ctx.enter_context(tc.tile_pool(name="sb", bufs=1))
        ps = ctx.enter_context(tc.tile_pool(name="ps", bufs=1, space="PSUM"))

        # ---- internal DRAM ----
        hrnT_loc = [dram.tile([D, NLH], FP8, name=f"hrnT_loc{h}")
                    for h in range(2)]
        hrn8_loc = dram.tile([NL, D], FP8)
        hrnT_all = [dram.tile([C, D, NLH], FP8, name=f"hrnT_all{h}",
                              addr_space="Shared") for h in range(2)]
        hrn8_all = dram.tile([C, BLK, D], FP8, addr_space="Shared")
        s_loc = dram.tile([M], F32)
        s_glob = dram.tile([M], F32, addr_space="Shared")
        vlr = dram.tile([M, D], FP8)
        vred = dram.tile([NL, D], FP8)

        # ---- SBUF resident ----
        exp_a = sb.tile([128, PB, M], FP8, name="exp_a")       # exp(a) rows
        hl_nb = sb.tile([128, PB, D], BF16, name="hl_nb")      # hl_n
        hl_nT = sb.tile([128, DK, NL], FP8, name="hl_nT")      # hl_n.T*SL
        hrn_b = sb.tile([128, PB, D], BF16, name="hrn_b")      # hr_n local
        hlp8 = sb.tile([128, PB, D], FP8, name="hlp8")         # hl'*S1 fp8
        hrn8c_st = sb.tile([128, 1, D], FP8, name="hrn8c_st")  # fp8 cast stage
        # streaming / staging (manual rotation via slot dims)
        rhsT = sb.tile([128, 4, DK, W1], FP8, name="rhsT")     # P1 rhs stream
        p0T_c = sb.tile([128, JB, G, 128], FP8, name="p0T_c")  # P2b lhsT cache
        vlr_st = sb.tile([128, 5, D], FP8, name="vlr_st")
        out_st = sb.tile([128, 3, DW], F32, name="out_st")
        vred_st = sb.tile([128, 2, DW], F32, name="vred_st")
        fin_st = sb.tile([128, 2, DW], F32, name="fin_st")
        trT_st = sb.tile([128, 2, DK, 128], FP8, name="trT_st")
        ld_st = sb.tile([128, 2, D], F32, name="ld_st")
        s_row = sb.tile([1, 1, W1], F32, name="s_row")
        # consts / stats
        ident_b = sb.tile([128, 128], BF16, name="ident_b")
        ident_e = sb.tile([128, 128], FP8, name="ident_e")
        ones_e = sb.tile([128, 2, 16], FP8, name="ones_e")
        stats = sb.tile([128, 352], F32, name="stats")
        r_parts = stats[:, 0:PB * JC].rearrange("p (a b) -> p a b", a=PB)
        r_red = stats[:, 128:128 + PB]
        r_red3 = stats[:, 128:128 + PB].rearrange("p (a b) -> p a b", b=1)
        rinv = stats[:, 136:136 + PB]
        s_sb = stats[:, 144:208][:, :JB]
        srec = stats[:, 208:272][:, :JB]
        sinv = stats[:, 272:336][:, :JB]
        nrm = stats[:, 336:352].rearrange("p (a b) -> p a b", a=8)  # [128,8,2]

        make_identity(nc, ident_b)
        nc.vector.tensor_copy(out=ident_e, in_=ident_b)
        nc.vector.memset(ones_e, 1.0)

        # ======= prep: hr first (gathers launch early), then hl ============
        SQUARE = mybir.ActivationFunctionType.Square

        def norm_chunk(src, dstb, pb, it):
            ld = ld_st[:, it % 2, :]
            nc.sync.dma_start(out=ld, in_=src[pb * 128:(pb + 1) * 128, :])
            nm = nrm[:, it % 8, :]
            for dd in range(DH):
                sq = ps.tile([128, DW], F32, tag="rot", bufs=3,
                             name=f"sq{it}_{dd}")
                nc.scalar.activation(out=sq, in_=ld[:, dd * DW:(dd + 1) * DW],
                                     func=SQUARE, accum_out=nm[:, dd:dd + 1])
            if DH > 1:
                nc.vector.tensor_add(out=nm[:, 0:1], in0=nm[:, 0:1],
                                     in1=nm[:, 1:2])
            nc.scalar.sqrt(out=nm[:, 1:2], in_=nm[:, 0:1])
            nc.vector.reciprocal(out=nm[:, 0:1], in_=nm[:, 1:2])
            nc.vector.tensor_scalar_mul(
                out=dstb[:, pb, :], in0=ld, scalar1=nm[:, 0:1])

        # interleaved prep: hr chunk (feeds gathers) then hl chunk — hl's
        # PE transposes fill the hr norm-chain latency gaps
        def hr_chunk(pb):
            h, pq = divmod(pb, PB // 2)
            norm_chunk(hr_in, hrn_b, pb, 2 * pb)
            tstage = trT_st[:, pb % 2, :, :]
            for dk in range(DK):
                pst = ps.tile([128, 128], BF16, tag="rot", bufs=3,
                              name=f"ptB{pb}_{dk}")
                nc.tensor.transpose(
                    pst, hrn_b[:, pb, dk * 128:(dk + 1) * 128], ident_b)
                nc.scalar.mul(out=tstage[:, dk, :], in_=pst, mul=SR)
            nc.sync.dma_start(
                out=hrnT_loc[h].rearrange("(dk p) j -> p dk j", p=128)
                [:, :, pq * 128:(pq + 1) * 128],
                in_=tstage)

        def hl_chunk(pb):
            norm_chunk(hl_in, hl_nb, pb, 2 * pb + 1)
            for dk in range(DK):
                pst = ps.tile([128, 128], BF16, tag="rot", bufs=3,
                              name=f"ptA{pb}_{dk}")
                nc.tensor.transpose(
                    pst, hl_nb[:, pb, dk * 128:(dk + 1) * 128], ident_b)
                nc.vector.tensor_scalar_mul(
                    out=hl_nT[:, dk, pb * 128:(pb + 1) * 128], in0=pst,
                    scalar1=SL)

        for pb in range(PB // 2):
            hr_chunk(pb)
        nc.gpsimd.collective_compute(
            "AllGather", BYPASS, replica_groups=groups,
            ins=[hrnT_loc[0].opt()], outs=[hrnT_all[0].opt()])
        for pb in range(PB // 2):
            hr_chunk(PB // 2 + pb)
            hl_chunk(2 * pb)
            hl_chunk(2 * pb + 1)
        nc.gpsimd.collective_compute(
            "AllGather", BYPASS, replica_groups=groups,
            ins=[hrnT_loc[1].opt()], outs=[hrnT_all[1].opt()])
        # hr_n fp8 -> DRAM + gather
        hrn8_rows = hrn8_loc.rearrange("(pb p) d -> p pb d", p=128)
        for pb in range(PB):
            t8 = hrn8c_st[:, 0, :]
            nc.gpsimd.tensor_copy(out=t8, in_=hrn_b[:, pb, :])
            nc.sync.dma_start(out=hrn8_rows[:, pb, :], in_=t8)
        nc.gpsimd.collective_compute(
            "AllGather", BYPASS, replica_groups=groups,
            ins=[hrn8_loc.opt()], outs=[hrn8_all.opt()])

        # ================= P1: a-matmul + exp + row/col sums ================
        # j-chunk order follows gather halves: (h, b, q)
        chunks = [(h, b, q) for h in range(2) for b in range(C)
                  for q in range(NQ)]
        if LVL >= 1:
            deferred = []

            def flush_deferred():
                while deferred:
                    deferred.pop(0)()

            psum_s = [None, None]

            for ci, (h, b, q) in enumerate(chunks):
                j0 = b * BLK + h * NLH + q * W1   # global j of this chunk
                rt = rhsT[:, ci % 4, :, :]
                nc.sync.dma_start(
                    out=rt,
                    in_=hrnT_all[h][b].rearrange("(dk p) j -> p dk j", p=128)
                    [:, :, q * W1:(q + 1) * W1])
                for ib in range(PB):
                    t_ = ci * PB + ib
                    pa = ps.tile([128, W1], F32,
                                 tag=("rot" if t_ % 7 < 3 else "acc"),
                                 bufs=(3 if t_ % 7 < 3 else 4),
                                 name=f"pa{ci}_{ib}")
                    for dkp in range(DK // 2):
                        nc.tensor.matmul(
                            pa,
                            lhsT=hl_nT[:, 2 * dkp:2 * dkp + 2,
                                       ib * 128:(ib + 1) * 128],
                            rhs=rt[:, 2 * dkp:2 * dkp + 2, :],
                            start=(dkp == 0), stop=(dkp == DK // 2 - 1),
                            perf_mode=DROW)
                    et = exp_a[:, ib, j0:j0 + W1]
                    nc.scalar.activation(
                        out=et, in_=pa, func=EXP, scale=1.0 / (SL * SR),
                        accum_out=r_parts[:, ib, ci:ci + 1])
                    if ib % 2 == 0:
                        flush_deferred()
                        continue

                    def ones_mm(ci=ci, ib=ib, j0=j0):
                        ibp = ib // 2
                        if ibp == 0:
                            psum_s[0] = ps.tile([1, W1], F32, tag="colsum",
                                                bufs=1, name=f"pscs{ci}")
                        # DoubleRow column-sum over an i-pair
                        nc.tensor.matmul(
                            psum_s[0], lhsT=ones_e[:, :, 0:1],
                            rhs=exp_a[:, ib - 1:ib + 1, j0:j0 + W1],
                            start=(ibp == 0), stop=(ibp == PB // 2 - 1),
                            perf_mode=DROW)
                        if ib == PB - 1:
                            sr = s_row[:, 0, :]
                            nc.vector.tensor_copy(out=sr, in_=psum_s[0])
                            nc.sync.dma_start(
                                out=s_loc[j0:j0 + W1].rearrange(
                                    "(a b) -> a b", a=1),
                                in_=sr)

                    flush_deferred()
                    deferred.append(ones_mm)
            flush_deferred()

            # r -> S1/r ; hl' fp8
            nc.vector.tensor_reduce(out=r_red3, in_=r_parts, op=ADD, axis=AXL_X)
            nc.vector.reciprocal(out=rinv, in_=r_red)
            for ib in range(PB):
                nc.vector.tensor_scalar(
                    out=hlp8[:, ib, :], in0=hl_nb[:, ib, :],
                    scalar1=rinv[:, ib:ib + 1], scalar2=S1, op0=MULT, op1=MULT)

            # col sums: AllReduce + readback in [p, jb] layout
            nc.gpsimd.collective_compute(
                "AllReduce", ADD, replica_groups=groups,
                ins=[s_loc.opt()], outs=[s_glob.opt()])
            nc.sync.dma_start(
                out=s_sb, in_=s_glob.rearrange("(b p) -> p b", p=128))
            nc.vector.reciprocal(out=srec, in_=s_sb)
            nc.vector.tensor_scalar_mul(out=sinv, in0=srec, scalar1=S2)

        # ================= P2a: v_lr = exp_a.T @ hl' -> ReduceScatter =======
        # P2b rhs kept resident per d-half; shares the rhsT slot (dead
        # after P1). Loaded once per half: 8 MB total instead of 32 MB
        # re-streamed, so P2b needs no DMA while the ReduceScatters run.
        rhs_res = sb.tile([128, JB, DW], FP8, name="rhs_res", tag="rhsT")

        def load_rhs_half(dh, engine=None):
            eng = engine or nc.gpsimd
            for b in range(C):
                ld_i = eng.dma_start(
                    out=rhs_res[:, b * (BLK // 128):(b + 1) * (BLK // 128), :],
                    in_=hrn8_all[b].rearrange("(jb p) d -> p jb d", p=128)
                    [:, :, dh * DW:(dh + 1) * DW])
                last_reload[0] = ld_i

        def transpose_grp(jb, g):
            ibs = list(range(g * G, min((g + 1) * G, PB)))
            for xi, ib in enumerate(ibs):
                # fp8 transpose: output element step must be 2
                pst = ps.tile([128, 128, 2], FP8, tag="rot", bufs=3,
                              name=f"ptb{g}_{jb}_{xi}")
                nc.tensor.transpose(
                    pst[:, :, 0],
                    exp_a[:, ib, jb * 128:(jb + 1) * 128], ident_e)
                if (jb * G + xi) % 3 == 2:
                    nc.scalar.activation(
                        out=p0T_c[:, jb, xi, :], in_=pst[:, :, 0],
                        func=COPY, scale=sinv[:, jb:jb + 1])
                else:
                    nc.vector.tensor_scalar_mul(
                        out=p0T_c[:, jb, xi, :], in0=pst[:, :, 0],
                        scalar1=sinv[:, jb:jb + 1])

        last_add = [None]
        last_add_dh = {}
        last_reload = [None]
        if LVL >= 2:
            if LVL >= 3:
                load_rhs_half(0)  # runs behind P2a's compute
            for jb in range(JB):
                st = vlr_st[:, jb % 5, :]
                pls = [ps.tile([128, DW], F32,
                               tag=("rot" if dh == 0 else "acc"),
                               bufs=(3 if dh == 0 else 4),
                               name=f"pl{jb}_{dh}") for dh in range(DH)]
                for icp in range(PB // 2):
                    for dh in range(DH):
                        # DoubleRow: contraction over an i-pair (K=256);
                        # dh-inner so consecutive matmuls share the lhsT load
                        nc.tensor.matmul(
                            pls[dh],
                            lhsT=exp_a[:, 2 * icp:2 * icp + 2,
                                       jb * 128:(jb + 1) * 128],
                            rhs=hlp8[:, 2 * icp:2 * icp + 2,
                                     dh * DW:(dh + 1) * DW],
                            start=(icp == 0), stop=(icp == PB // 2 - 1),
                            perf_mode=DROW)
                for dh in range(DH):
                    nc.scalar.activation(out=st[:, dh * DW:(dh + 1) * DW],
                                         in_=pls[dh], func=COPY,
                                         scale=SV / S1)
                nc.scalar.dma_start(
                    out=vlr[jb * 128:(jb + 1) * 128, :], in_=st)
            # single ReduceScatter runs on the CC engine while P2b
            # (which needs no DMA) computes
            nc.gpsimd.collective_compute(
                "ReduceScatter", ADD, replica_groups=groups,
                ins=[vlr.opt()], outs=[vred.opt()])

        # ====== P2b: mu_rl = hl_n - (p0T.T @ hr_n)/S2  (sweeps dh x g) ======
        if LVL >= 3:
            for dh in range(DH):
                if dh > 0:
                    # HW lanes: per-b-block deps let dh1 sweeps start as
                    # soon as the first block lands
                    load_rhs_half(dh, engine=nc.sync)
                for g in range(NG):
                    ibs = list(range(g * G, min((g + 1) * G, PB)))
                    accs = [ps.tile([128, DW], F32, tag="acc", bufs=4,
                                    name=f"acc{g}_{dh}_{xi}")
                            for xi in range(len(ibs))]
                    if dh == 0:
                        for jb in range(4):
                            transpose_grp(jb, g)
                    for jbp in range(JB // 2):
                        if dh == 0:
                            for jb in (2 * jbp + 4, 2 * jbp + 5):
                                if jb < JB:
                                    transpose_grp(jb, g)
                        for xi in range(len(ibs)):
                            # DoubleRow: contraction over a jb-pair (K=256)
                            nc.tensor.matmul(
                                accs[xi],
                                lhsT=p0T_c[:, 2 * jbp:2 * jbp + 2, xi, :],
                                rhs=rhs_res[:, 2 * jbp:2 * jbp + 2, :],
                                start=(jbp == 0), stop=(jbp == JB // 2 - 1),
                                perf_mode=DROW)
                    for xi, ib in enumerate(ibs):
                        st = out_st[:, ((dh * NG + g) * G + xi) % 3, :DW]
                        nc.scalar.activation(
                            out=st, in_=accs[xi], func=COPY, scale=-1.0 / S2)
                        last_add[0] = nc.vector.tensor_add(
                            out=st, in0=st,
                            in1=hl_nb[:, ib, dh * DW:(dh + 1) * DW])
                        last_add_dh[dh] = last_add[0]
                        nc.gpsimd.dma_start(
                            out=mu_rl_o[ib * 128:(ib + 1) * 128,
                                        dh * DW:(dh + 1) * DW], in_=st)

        # ================= final: mu_lr = hr_n - vred (on gpsimd) ===========
        if LVL >= 4:
            for dh in range(DH):
                for pb in range(PB):
                    vs = vred_st[:, pb % 2, :]
                    rd_i = nc.gpsimd.dma_start(
                        out=vs,
                        in_=vred[pb * 128:(pb + 1) * 128,
                                 dh * DW:(dh + 1) * DW])
                    if last_reload[0] is not None:
                        add_dep_helper(rd_i.ins, last_reload[0].ins,
                                       sync=False, reason="after reload")
                    st = fin_st[:, pb % 2, :]
                    nc.scalar.activation(
                        out=st, in_=vs, func=COPY, scale=-1.0 / SV)
                    sub_i = nc.vector.tensor_add(
                        out=st, in0=st,
                        in1=hrn_b[:, pb, dh * DW:(dh + 1) * DW])
                    pin = last_add_dh.get(dh, last_add[0])
                    if pin is not None:
                        add_dep_helper(sub_i.ins, pin.ins,
                                       sync=False, reason="after P2b adds")
                    nc.sync.dma_start(
                        out=mu_lr_o[pb * 128:(pb + 1) * 128,
                                    dh * DW:(dh + 1) * DW], in_=st)

        # dummy writes for any output a stopped-early build didn't produce
        if LVL < 4:
            for pb in range(PB):
                for dd in range(DH):
                    st = out_st[:, pb % 3, :]
                    nc.vector.tensor_copy(
                        out=st, in_=hrn_b[:, pb, dd * DW:(dd + 1) * DW])
                    nc.sync.dma_start(
                        out=mu_lr_o[pb * 128:(pb + 1) * 128,
                                    dd * DW:(dd + 1) * DW], in_=st)
        if LVL < 3:
            for pb in range(PB):
                for dh in range(DH):
                    st = fin_st[:, pb % 2, :]
                    nc.vector.tensor_copy(
                        out=st, in_=hl_nb[:, pb, dh * DW:(dh + 1) * DW])
                    nc.sync.dma_start(
                        out=mu_rl_o[pb * 128:(pb + 1) * 128,
                                    dh * DW:(dh + 1) * DW], in_=st)

    nc.compile()
    return nc


_NC_CACHE = {}


def _get_nc():
    if "nc" not in _NC_CACHE:
        _NC_CACHE["nc"] = build(C=8, NL=1024, M=8192, D=1024)
    return _NC_CACHE["nc"]


def kernel(hl, hr):
    """Full inputs in, full outputs out; distributes across 8 cores."""
    from concourse.bass_utils import run_bass_kernel_spmd

    C, NL = 8, 1024
    hl = np.ascontiguousarray(np.asarray(hl, dtype=np.float32))
    hr = np.ascontiguousarray(np.asarray(hr, dtype=np.float32))
    nc = _get_nc()
    in_maps = [
        {"hl": np.ascontiguousarray(hl[c * NL:(c + 1) * NL]),
         "hr": np.ascontiguousarray(hr[c * NL:(c + 1) * NL])}
        for c in range(C)
    ]
    res = run_bass_kernel_spmd(nc, in_maps, list(range(C)))
    mu_lr = np.concatenate([res.results[c]["mu_lr"] for c in range(C)])
    mu_rl = np.concatenate([res.results[c]["mu_rl"] for c in range(C)])
    return mu_lr, mu_rl

